# revision 20
# baseline (speedup 1.0000x reference)
"""Trainium2 Bass kernel for nn_CHGANSimplified (sparse graph attention).

Math (per batch b, time t):
  enh = x + type_embed[parity(n)]
  Q/K/V = enh @ W*.T + b*          (4 heads, head dim 32)
  S_h = (Q_h K_h^T)/sqrt(32) + edge_bias ; masked where adj==0 & ~eye
  out = LN(concat_h(softmax(S_h) V_h) @ Wo.T + bo + x)

Device strategy (8 cores, data-parallel over the 24 (b,t) pairs, 3 each).
Per-core schedule (the ACT engine's exp stream is the critical resource):

  - feature-major layout: enhT/qt/kt (D=128 part, N=1024); head h sits
    at partitions 32h..32h+31.
  - QK^T: row-tiled PE (K=32/head, tile_position=(32h,0)); exp off PSUM
    on ACT -> e bf16; multiplicative mask (0/1 incl diag), one DVE mul
    per [128,1024] chunk via a column-doubled mask layout.
  - AV: col-tiled PE (V_h natural [m,32] stationary, tile (0,32h)),
    feature-major output; denominator via all-ones [128,32] stationary
    in the same pass (per-lane broadcast rowsums); fast-reciprocal+mul.
  - Wo: stationary-swap (evT chunk stationary) -> natural output for
    residual + LayerNorm; LN sqrt deferred to one tail (one ACT table
    switch total).
  - PE runs in-order and mode switches (row<->col tiling) cost ~650ns,
    so AV/proj/Wo blocks are fenced behind a PE nop that reads the last
    QK st tile of the surrounding exp window: each block executes as
    one solid excursion in the window's tail instead of thrashing modes.
"""

import os
import sys

sys.path.insert(0, "/opt/trn_rl_repo")

from contextlib import ExitStack

import ml_dtypes
import numpy as np

import concourse.bass as bass
import concourse.tile as tile
from concourse import bacc, mybir
from concourse.bass_utils import run_bass_kernel_spmd

B, N, T, D, H, DH = 2, 1024, 12, 128, 4, 32
NCORES = 8
PAIRS = [(b, t) for b in range(B) for t in range(T)]
PER_CORE = len(PAIRS) // NCORES  # 3
EPS = 1e-5
NTILE = N // 128  # 8

BF16, BF16_NP = mybir.dt.bfloat16, ml_dtypes.bfloat16
F32 = mybir.dt.float32
AF = mybir.ActivationFunctionType

# number of (m,hp) mask-mul chunks per (pair,j) stage routed to gpsimd
POOLM = int(os.environ.get("BASSK_POOLM", "0"))
# chunk index within a stage_A window at which the fenced excursion runs
FENCE_AT = int(os.environ.get("BASSK_FENCE", "6"))

LAST_RESULTS = None  # BassKernelResults of the most recent run (for test.py)

CBF_W = 4 * 128 + DH  # wq|wk|wv|wo|ones32
CF_W = 1 + 1 + 3 * 128  # bq|bk|bvb|lng|lnb


def _build_nc():
    nc = bacc.Bacc()

    xT_d = nc.dram_tensor("xt", [PER_CORE, 128, N], F32, kind="ExternalInput")
    # host pre-permuted to [128, q, d] so the DMA is contiguous per partition
    xpb_d = nc.dram_tensor("xpb", [PER_CORE, 128, NTILE * D], F32, kind="ExternalInput")
    ta_d = nc.dram_tensor("ta", [128, N], F32, kind="ExternalInput")
    cbf_d = nc.dram_tensor("cbf", [128, CBF_W], BF16, kind="ExternalInput")
    cf_d = nc.dram_tensor("cf", [128, CF_W], F32, kind="ExternalInput")
    maskd_d = nc.dram_tensor("maskd", [N, 2 * N], BF16, kind="ExternalInput")
    out_d = nc.dram_tensor("out", [PER_CORE, 128, NTILE * D], F32, kind="ExternalOutput")

    with tile.TileContext(nc) as tc, ExitStack() as ctx:
        const = ctx.enter_context(tc.tile_pool(name="const", bufs=1))
        work = ctx.enter_context(tc.tile_pool(name="work", bufs=2))
        expp = ctx.enter_context(tc.tile_pool(name="expp", bufs=40))
        pst = ctx.enter_context(tc.tile_pool(name="pst", bufs=3, space="PSUM"))
        pdp = ctx.enter_context(tc.tile_pool(name="pdp", bufs=1, space="PSUM"))

        # ---- constants (two consolidated blobs + ta + masks) ----
        ta_sb = const.tile([128, N], F32)
        nc.gpsimd.dma_start(ta_sb, ta_d[:, :])
        cbf_sb = const.tile([128, CBF_W], BF16)
        nc.gpsimd.dma_start(cbf_sb, cbf_d[:, :])
        cf_sb = const.tile([128, CF_W], F32)
        nc.gpsimd.dma_start(cf_sb, cf_d[:, :])
        wq_sb = cbf_sb[:, 0:128]
        wk_sb = cbf_sb[:, 128:256]
        wv_sb = cbf_sb[:, 256:384]
        wo_sb = cbf_sb[:, 384:512]
        ones_sb = cbf_sb[:, 512 : 512 + DH]
        bq_sb = cf_sb[:, 0:1]
        bk_sb = cf_sb[:, 1:2]
        bvb_sb = cf_sb[:, 2:130]
        lng_sb = cf_sb[:, 130:258]
        lnb_sb = cf_sb[:, 258:386]
        eps_sb = const.tile([128, 1], F32)
        nc.vector.memset(eps_sb, EPS)
        mask_sb = []
        for m in range(NTILE):
            mt = const.tile([128, 2 * N], BF16, name=f"mask{m}", tag=f"mask{m}")
            nc.gpsimd.dma_start(mt, maskd_d[m * 128 : (m + 1) * 128, :])
            mask_sb.append(mt)

        qts, kts, vbs, evs, xpbs, ys, mvs, es = {}, {}, {}, {}, {}, {}, {}, {}
        last_st = [None]

        def pe_fence():
            """PE nop reading the newest st tile: blocks later PE work
            (mode-switching excursions) until the current exp window's QK
            stream has fully issued, so excursions run as one solid block."""
            if last_st[0] is None:
                return
            with tc.tile_critical():
                nop = nc.tensor.nop(hint="dep", nofuse=True).ins
                nop.ins = [nc.tensor.lower_ap(last_st[0][:, 0:1])]

        def load_xpb(it):
            xpb_sb = work.tile([128, NTILE, D], F32, name=f"xpb{it}", tag="xpb", bufs=2)
            nc.sync.dma_start(xpb_sb, xpb_d[it].rearrange("p (q d) -> p q d", q=NTILE))
            xpbs[it] = xpb_sb

        def stage_P(it, with_xpb=False):
            """load + enhance + Q/K/V projections for pair `it`."""
            xT_sb = work.tile([128, N], F32, name=f"xT{it}", tag="xT", bufs=2)
            nc.sync.dma_start(xT_sb, xT_d[it])
            if with_xpb:
                load_xpb(it)
            enhT = work.tile([128, N], BF16, name=f"enhT{it}", tag="enhT", bufs=2)
            nc.vector.tensor_add(enhT, xT_sb, ta_sb)

            for nm, w_sb, b_sb in (("q", wq_sb, bq_sb), ("k", wk_sb, bk_sb)):
                ps = pst.tile([128, N], F32, name=f"ps{nm}{it}", tag="st")
                for j in range(2):
                    nc.tensor.matmul(
                        ps[:, j * 512 : (j + 1) * 512],
                        w_sb,
                        enhT[:, j * 512 : (j + 1) * 512],
                        start=True,
                        stop=True,
                    )
                dst = work.tile([128, N], BF16, name=f"{nm}t{it}", tag=f"{nm}t", bufs=2)
                nc.vector.tensor_scalar_add(dst, ps, b_sb)
                if nm == "q":
                    qts[it] = dst
                else:
                    kts[it] = dst

            vb = work.tile([128, NTILE, D], BF16, name=f"vb{it}", tag="vb", bufs=2)
            for m in range(NTILE):
                psv = pst.tile([128, D], F32, name=f"psv{it}_{m}", tag="st")
                nc.tensor.matmul(
                    psv, enhT[:, m * 128 : (m + 1) * 128], wv_sb, start=True, stop=True
                )
                nc.vector.tensor_add(vb[:, m, :], psv, bvb_sb)
            vbs[it] = vb
            evs[it] = work.tile([128, N], BF16, name=f"ev{it}", tag="ev", bufs=2)

        def stage_A(it, j, hidden=()):
            """QK^T + exp + mask for nq half `j` of pair `it`; the fenced
            excursion blocks are emitted mid-window (chunk FENCE_AT) so
            they finish inside this exp window instead of stalling the
            next one."""
            qt, kt = qts[it], kts[it]
            chunk = 0
            for m in range(NTILE):
                for hp in range(2):
                    st = pst.tile([128, N], F32, name=f"st{it}_{j}_{m}_{hp}", tag="st")
                    for hh in range(2):
                        h = 2 * hp + hh
                        nc.tensor.matmul(
                            st[:, hh * 512 : (hh + 1) * 512],
                            kt[32 * h : 32 * h + 32, m * 128 : (m + 1) * 128],
                            qt[32 * h : 32 * h + 32, j * 512 : (j + 1) * 512],
                            start=True,
                            stop=True,
                            tile_position=(32 * h, 0),
                        )
                    last_st[0] = st
                    e = expp.tile([128, N], BF16, name=f"e{it}_{j}_{m}_{hp}", tag="e")
                    nc.scalar.activation(e, st, AF.Exp)
                    eng = nc.gpsimd if chunk < POOLM else nc.vector
                    eng.tensor_mul(e, e, mask_sb[m][:, j * N : (j + 1) * N])
                    es[(it, j, m, hp)] = e
                    chunk += 1
                    if chunk == FENCE_AT and hidden:
                        pe_fence()
                        for fn in hidden:
                            fn()
            if FENCE_AT >= 16 and hidden:
                pe_fence()
                for fn in hidden:
                    fn()

        def stage_B(it, j):
            """col-tiled AV + denominator + normalize for (pair, j)."""
            vb, ev = vbs[it], evs[it]
            pd = pdp.tile([128, N], F32, name=f"pd{it}_{j}", tag="pd")
            # all AV matmuls first, then all denominator matmuls: the ones
            # stationary then loads once per col tile instead of ping-ponging
            # with V chunks every matmul
            for m in range(NTILE):
                for h in range(H):
                    e = es[(it, j, m, h // 2)]
                    nc.tensor.matmul(
                        pd[32 * h : 32 * h + 32, 0:512],
                        vb[:, m, 32 * h : 32 * h + 32],
                        e[:, (h % 2) * 512 : (h % 2) * 512 + 512],
                        start=(m == 0),
                        stop=(m == NTILE - 1),
                        tile_position=(0, 32 * h),
                    )
            for m in range(NTILE):
                for h in range(H):
                    e = es[(it, j, m, h // 2)]
                    nc.tensor.matmul(
                        pd[32 * h : 32 * h + 32, 512:1024],
                        ones_sb,
                        e[:, (h % 2) * 512 : (h % 2) * 512 + 512],
                        start=(m == 0),
                        stop=(m == NTILE - 1),
                        tile_position=(0, 32 * h),
                    )
            rec = work.tile([128, 512], F32, name=f"rec{it}_{j}", tag="rec", bufs=2)
            nc.vector.reciprocal_approx_fast(rec, pd[:, 512:1024])
            nc.vector.tensor_mul(ev[:, j * 512 : (j + 1) * 512], pd[:, 0:512], rec)

        def stage_O(it):
            """Wo projection (stationary-swap -> natural) + residual + stats."""
            ev, xpb_sb = evs[it], xpbs[it]
            y = work.tile([128, NTILE, D], F32, name=f"y{it}", tag=f"y{it}", bufs=1)
            mv = work.tile([128, NTILE, 2], F32, name=f"mv{it}", tag=f"mv{it}", bufs=1)
            for c in range(NTILE):
                pso = pst.tile([128, D], F32, name=f"pso{it}_{c}", tag="st")
                nc.tensor.matmul(
                    pso, ev[:, c * 128 : (c + 1) * 128], wo_sb, start=True, stop=True
                )
                nc.vector.tensor_add(y[:, c, :], pso, xpb_sb[:, c, :])
                st6 = work.tile([128, 6], F32, name=f"st6{it}_{c}", tag="st6", bufs=8)
                nc.vector.bn_stats(st6, y[:, c, :])
                nc.vector.bn_aggr(mv[:, c, :], st6)
            ys[it], mvs[it] = y, mv

        def stage_LN(it):
            """LayerNorm + store; rstd = exp(-0.5*ln(var+eps)) keeps the
            ACT table on the natural_log_exp set (no sqrt table switch)."""
            y, mv = ys[it], mvs[it]
            # bias tile derived from the last e tile: pins the sqrt (and its
            # ACT table switch) behind the final exp of the stream
            e_last = es[(PER_CORE - 1, 1, NTILE - 1, 1)]
            epsl = work.tile([128, 1], F32, name=f"epsl{it}", tag="epsl", bufs=2)
            nc.vector.tensor_scalar(
                epsl,
                e_last[:, 0:1],
                0.0,
                EPS,
                op0=mybir.AluOpType.mult,
                op1=mybir.AluOpType.add,
            )
            sd = work.tile([128, NTILE, 1], F32, name=f"sd{it}", tag="sd", bufs=2)
            nc.scalar.activation(sd, mv[:, :, 1:2], AF.Sqrt, bias=epsl[:, 0:1])
            rstd = work.tile([128, NTILE, 1], F32, name=f"rstd{it}", tag="rstd", bufs=2)
            nc.vector.reciprocal(rstd, sd)
            oall = work.tile([128, NTILE, D], F32, name=f"oall{it}", tag="oall", bufs=2)
            for c in range(NTILE):
                z = work.tile([128, D], F32, name=f"z{it}_{c}", tag="z", bufs=4)
                nc.vector.tensor_scalar(
                    z,
                    y[:, c, :],
                    mv[:, c, 0:1],
                    rstd[:, c, 0:1],
                    op0=mybir.AluOpType.subtract,
                    op1=mybir.AluOpType.mult,
                )
                nc.vector.tensor_mul(z, z, lng_sb)
                nc.vector.tensor_add(oall[:, c, :], z, lnb_sb)
            nc.sync.dma_start(out_d[it].rearrange("p (q d) -> p q d", q=NTILE), oall)

        # ---- half-pair software pipeline with fenced excursions ----
        stage_P(0)
        stage_A(0, 0, [lambda: stage_P(1), lambda: load_xpb(0)])
        stage_A(0, 1, [lambda: stage_B(0, 0), lambda: load_xpb(1)])
        stage_A(1, 0, [lambda: stage_B(0, 1), lambda: stage_O(0)])
        stage_A(1, 1, [lambda: stage_B(1, 0), lambda: stage_P(2, True)])
        stage_A(2, 0, [lambda: stage_B(1, 1), lambda: stage_O(1)])
        stage_A(2, 1, [lambda: stage_B(2, 0)])
        stage_B(2, 1)
        stage_O(2)
        for it in range(PER_CORE):
            stage_LN(it)

    nc.compile()
    return nc


_nc_cache = {}


def _get_nc():
    key = (POOLM, FENCE_AT)
    if key not in _nc_cache:
        _nc_cache[key] = _build_nc()
    return _nc_cache[key]


def kernel(
    node_features,
    adj_mx,
    node_type_embed,
    Wq,
    bq,
    Wk,
    bk,
    Wv,
    bv,
    edge_bias,
    Wo,
    bo,
    ln_g,
    ln_b,
):
    global LAST_RESULTS
    nf = np.asarray(node_features, np.float32)
    adj = np.asarray(adj_mx)
    nte = np.asarray(node_type_embed, np.float32)
    Wq = np.asarray(Wq, np.float32)
    Wk = np.asarray(Wk, np.float32)
    Wv = np.asarray(Wv, np.float32)
    Wo = np.asarray(Wo, np.float32)
    bq = np.asarray(bq, np.float32)
    bk = np.asarray(bk, np.float32)
    bv = np.asarray(bv, np.float32)
    bo = np.asarray(bo, np.float32)
    edge_bias = np.asarray(edge_bias, np.float32)
    ln_g = np.asarray(ln_g, np.float32)
    ln_b = np.asarray(ln_b, np.float32)

    scale = 1.0 / np.sqrt(DH)

    # shared (replicated) inputs
    types = 1 - (np.arange(N) % 2)
    ta = np.ascontiguousarray(nte[types].T)  # (D, N)
    keep = np.maximum(adj.astype(np.float32), np.eye(N, dtype=np.float32))
    mm = (np.exp(edge_bias) * keep).T.astype(BF16_NP)  # (m, nq)
    # column-doubled mask: [j0 | j0 | j1 | j1] so one [128,1024] mul covers
    # both heads of a pair (e layout is [h0 512 | h1 512] per nq half)
    maskd = np.concatenate(
        [mm[:, 0:512], mm[:, 0:512], mm[:, 512:1024], mm[:, 512:1024]], axis=1
    )
    cbf = np.concatenate(
        [
            (Wq.T * scale).astype(BF16_NP),
            Wk.T.astype(BF16_NP),
            Wv.T.astype(BF16_NP),
            Wo.T.astype(BF16_NP),
            np.ones((128, DH), BF16_NP),
        ],
        axis=1,
    )
    cf = np.concatenate(
        [
            (bq * scale).reshape(D, 1),
            bk.reshape(D, 1),
            np.broadcast_to(bv, (128, D)),
            np.broadcast_to(ln_g, (128, D)),
            np.broadcast_to(ln_b, (128, D)),
        ],
        axis=1,
    ).astype(np.float32)
    shared = {
        "ta": ta,
        "cbf": np.ascontiguousarray(cbf),
        "cf": np.ascontiguousarray(cf),
        "maskd": np.ascontiguousarray(maskd),
    }

    in_maps = []
    for c in range(NCORES):
        pairs = PAIRS[c * PER_CORE : (c + 1) * PER_CORE]
        xT = np.stack([np.ascontiguousarray(nf[b, :, t, :].T) for (b, t) in pairs])
        # [q*128+p, d] -> [p, q*d] so the device DMA is contiguous/partition
        xpb = np.stack(
            [
                (nf[b, :, t, :] + bo)
                .reshape(NTILE, 128, D)
                .transpose(1, 0, 2)
                .reshape(128, NTILE * D)
                for (b, t) in pairs
            ]
        )
        in_maps.append({**shared, "xt": xT, "xpb": np.ascontiguousarray(xpb)})

    nc = _get_nc()
    res = run_bass_kernel_spmd(
        nc,
        in_maps,
        core_ids=list(range(NCORES)),
        trace=bool(int(os.environ.get("BASSK_TRACE", "0"))),
    )
    LAST_RESULTS = res

    out = np.empty((B, N, T, D), np.float32)
    for c in range(NCORES):
        pairs = PAIRS[c * PER_CORE : (c + 1) * PER_CORE]
        for i, (b, t) in enumerate(pairs):
            o = res.results[c]["out"][i].reshape(128, NTILE, D)
            out[b, :, t, :] = o.transpose(1, 0, 2).reshape(N, D)
    return out


# revision 22
# speedup vs baseline: 1.0092x; 1.0092x over previous
"""Trainium2 Bass kernel for nn_CHGANSimplified (sparse graph attention).

Math (per batch b, time t):
  enh = x + type_embed[parity(n)]
  Q/K/V = enh @ W*.T + b*          (4 heads, head dim 32)
  S_h = (Q_h K_h^T)/sqrt(32) + edge_bias ; masked where adj==0 & ~eye
  out = LN(concat_h(softmax(S_h) V_h) @ Wo.T + bo + x)

Device strategy (8 cores, data-parallel over the 24 (b,t) pairs, 3 each).
Per-core schedule (the ACT engine's exp stream is the critical resource):

  - feature-major layout: enhT/qt/kt (D=128 part, N=1024); head h sits
    at partitions 32h..32h+31.
  - QK^T: row-tiled PE (K=32/head, tile_position=(32h,0)); exp off PSUM
    on ACT -> e bf16; multiplicative mask (0/1 incl diag), one DVE mul
    per [128,1024] chunk via a column-doubled mask layout.
  - AV: col-tiled PE (V_h natural [m,32] stationary, tile (0,32h)),
    feature-major output; denominator via all-ones [128,32] stationary
    in the same pass (per-lane broadcast rowsums); fast-reciprocal+mul.
  - Wo: stationary-swap (evT chunk stationary) -> natural output for
    residual + LayerNorm; LN sqrt deferred to one tail (one ACT table
    switch total).
  - PE runs in-order and mode switches (row<->col tiling) cost ~650ns,
    so AV/proj/Wo blocks are fenced behind a PE nop that reads the last
    QK st tile of the surrounding exp window: each block executes as
    one solid excursion in the window's tail instead of thrashing modes.
"""

import os
import sys

sys.path.insert(0, "/opt/trn_rl_repo")

from contextlib import ExitStack

import ml_dtypes
import numpy as np

import concourse.bass as bass
import concourse.tile as tile
from concourse import bacc, mybir
from concourse.bass_utils import run_bass_kernel_spmd

B, N, T, D, H, DH = 2, 1024, 12, 128, 4, 32
NCORES = 8
PAIRS = [(b, t) for b in range(B) for t in range(T)]
PER_CORE = len(PAIRS) // NCORES  # 3
EPS = 1e-5
NTILE = N // 128  # 8

BF16, BF16_NP = mybir.dt.bfloat16, ml_dtypes.bfloat16
F32 = mybir.dt.float32
AF = mybir.ActivationFunctionType

# number of (m,hp) mask-mul chunks per (pair,j) stage routed to gpsimd
POOLM = int(os.environ.get("BASSK_POOLM", "2"))
# chunk index within a stage_A window at which the fenced excursion runs
FENCE_AT = int(os.environ.get("BASSK_FENCE", "9"))

LAST_RESULTS = None  # BassKernelResults of the most recent run (for test.py)

CBF_W = 4 * 128 + DH  # wq|wk|wv|wo|ones32
CF_W = 1 + 1 + 3 * 128  # bq|bk|bvb|lng|lnb


def _build_nc():
    nc = bacc.Bacc()

    xT_d = nc.dram_tensor("xt", [PER_CORE, 128, N], F32, kind="ExternalInput")
    # host pre-permuted to [128, q, d] so the DMA is contiguous per partition
    xpb_d = nc.dram_tensor("xpb", [PER_CORE, 128, NTILE * D], F32, kind="ExternalInput")
    ta_d = nc.dram_tensor("ta", [128, N], F32, kind="ExternalInput")
    cbf_d = nc.dram_tensor("cbf", [128, CBF_W], BF16, kind="ExternalInput")
    cf_d = nc.dram_tensor("cf", [128, CF_W], F32, kind="ExternalInput")
    maskd_d = nc.dram_tensor("maskd", [N, 2 * N], BF16, kind="ExternalInput")
    out_d = nc.dram_tensor("out", [PER_CORE, 128, NTILE * D], F32, kind="ExternalOutput")

    with tile.TileContext(nc) as tc, ExitStack() as ctx:
        const = ctx.enter_context(tc.tile_pool(name="const", bufs=1))
        work = ctx.enter_context(tc.tile_pool(name="work", bufs=2))
        expp = ctx.enter_context(tc.tile_pool(name="expp", bufs=40))
        pst = ctx.enter_context(tc.tile_pool(name="pst", bufs=3, space="PSUM"))
        pdp = ctx.enter_context(tc.tile_pool(name="pdp", bufs=1, space="PSUM"))

        # ---- constants (two consolidated blobs + ta + masks) ----
        ta_sb = const.tile([128, N], F32)
        nc.gpsimd.dma_start(ta_sb, ta_d[:, :])
        cbf_sb = const.tile([128, CBF_W], BF16)
        nc.gpsimd.dma_start(cbf_sb, cbf_d[:, :])
        cf_sb = const.tile([128, CF_W], F32)
        nc.gpsimd.dma_start(cf_sb, cf_d[:, :])
        wq_sb = cbf_sb[:, 0:128]
        wk_sb = cbf_sb[:, 128:256]
        wv_sb = cbf_sb[:, 256:384]
        wo_sb = cbf_sb[:, 384:512]
        ones_sb = cbf_sb[:, 512 : 512 + DH]
        bq_sb = cf_sb[:, 0:1]
        bk_sb = cf_sb[:, 1:2]
        bvb_sb = cf_sb[:, 2:130]
        lng_sb = cf_sb[:, 130:258]
        lnb_sb = cf_sb[:, 258:386]
        eps_sb = const.tile([128, 1], F32)
        nc.vector.memset(eps_sb, EPS)
        mask_sb = []
        for m in range(NTILE):
            mt = const.tile([128, 2 * N], BF16, name=f"mask{m}", tag=f"mask{m}")
            nc.gpsimd.dma_start(mt, maskd_d[m * 128 : (m + 1) * 128, :])
            mask_sb.append(mt)

        qts, kts, vbs, evs, xpbs, ys, mvs, es = {}, {}, {}, {}, {}, {}, {}, {}
        last_st = [None]

        def pe_fence():
            """PE nop reading the newest st tile: blocks later PE work
            (mode-switching excursions) until the current exp window's QK
            stream has fully issued, so excursions run as one solid block."""
            if last_st[0] is None:
                return
            with tc.tile_critical():
                nop = nc.tensor.nop(hint="dep", nofuse=True).ins
                nop.ins = [nc.tensor.lower_ap(last_st[0][:, 0:1])]

        def load_xpb(it):
            xpb_sb = work.tile([128, NTILE, D], F32, name=f"xpb{it}", tag="xpb", bufs=2)
            nc.sync.dma_start(xpb_sb, xpb_d[it].rearrange("p (q d) -> p q d", q=NTILE))
            xpbs[it] = xpb_sb

        def stage_P(it, with_xpb=False):
            """load + enhance + Q/K/V projections for pair `it`."""
            xT_sb = work.tile([128, N], F32, name=f"xT{it}", tag="xT", bufs=2)
            nc.sync.dma_start(xT_sb, xT_d[it])
            if with_xpb:
                load_xpb(it)
            enhT = work.tile([128, N], BF16, name=f"enhT{it}", tag="enhT", bufs=2)
            nc.vector.tensor_add(enhT, xT_sb, ta_sb)

            for nm, w_sb, b_sb in (("q", wq_sb, bq_sb), ("k", wk_sb, bk_sb)):
                ps = pst.tile([128, N], F32, name=f"ps{nm}{it}", tag="st")
                for j in range(2):
                    nc.tensor.matmul(
                        ps[:, j * 512 : (j + 1) * 512],
                        w_sb,
                        enhT[:, j * 512 : (j + 1) * 512],
                        start=True,
                        stop=True,
                    )
                dst = work.tile([128, N], BF16, name=f"{nm}t{it}", tag=f"{nm}t", bufs=2)
                nc.vector.tensor_scalar_add(dst, ps, b_sb)
                if nm == "q":
                    qts[it] = dst
                else:
                    kts[it] = dst

            vb = work.tile([128, NTILE, D], BF16, name=f"vb{it}", tag="vb", bufs=2)
            for m in range(NTILE):
                psv = pst.tile([128, D], F32, name=f"psv{it}_{m}", tag="st")
                nc.tensor.matmul(
                    psv, enhT[:, m * 128 : (m + 1) * 128], wv_sb, start=True, stop=True
                )
                nc.vector.tensor_add(vb[:, m, :], psv, bvb_sb)
            vbs[it] = vb
            evs[it] = work.tile([128, N], BF16, name=f"ev{it}", tag="ev", bufs=2)

        def stage_A(it, j, hidden=()):
            """QK^T + exp + mask for nq half `j` of pair `it`; the fenced
            excursion blocks are emitted mid-window (chunk FENCE_AT) so
            they finish inside this exp window instead of stalling the
            next one."""
            qt, kt = qts[it], kts[it]
            chunk = 0
            for m in range(NTILE):
                for hp in range(2):
                    st = pst.tile([128, N], F32, name=f"st{it}_{j}_{m}_{hp}", tag="st")
                    for hh in range(2):
                        h = 2 * hp + hh
                        nc.tensor.matmul(
                            st[:, hh * 512 : (hh + 1) * 512],
                            kt[32 * h : 32 * h + 32, m * 128 : (m + 1) * 128],
                            qt[32 * h : 32 * h + 32, j * 512 : (j + 1) * 512],
                            start=True,
                            stop=True,
                            tile_position=(32 * h, 0),
                        )
                    last_st[0] = st
                    e = expp.tile([128, N], BF16, name=f"e{it}_{j}_{m}_{hp}", tag="e")
                    nc.scalar.activation(e, st, AF.Exp)
                    eng = nc.gpsimd if chunk < POOLM else nc.vector
                    eng.tensor_mul(e, e, mask_sb[m][:, j * N : (j + 1) * N])
                    es[(it, j, m, hp)] = e
                    chunk += 1
                    if chunk == FENCE_AT and hidden:
                        pe_fence()
                        for fn in hidden:
                            fn()
            if FENCE_AT >= 16 and hidden:
                pe_fence()
                for fn in hidden:
                    fn()

        def stage_B(it, j):
            """col-tiled AV + denominator + normalize for (pair, j)."""
            vb, ev = vbs[it], evs[it]
            pd = pdp.tile([128, N], F32, name=f"pd{it}_{j}", tag="pd")
            # all AV matmuls first, then all denominator matmuls: the ones
            # stationary then loads once per col tile instead of ping-ponging
            # with V chunks every matmul
            for m in range(NTILE):
                for h in range(H):
                    e = es[(it, j, m, h // 2)]
                    nc.tensor.matmul(
                        pd[32 * h : 32 * h + 32, 0:512],
                        vb[:, m, 32 * h : 32 * h + 32],
                        e[:, (h % 2) * 512 : (h % 2) * 512 + 512],
                        start=(m == 0),
                        stop=(m == NTILE - 1),
                        tile_position=(0, 32 * h),
                    )
            for m in range(NTILE):
                for h in range(H):
                    e = es[(it, j, m, h // 2)]
                    nc.tensor.matmul(
                        pd[32 * h : 32 * h + 32, 512:1024],
                        ones_sb,
                        e[:, (h % 2) * 512 : (h % 2) * 512 + 512],
                        start=(m == 0),
                        stop=(m == NTILE - 1),
                        tile_position=(0, 32 * h),
                    )
            rec = work.tile([128, 512], F32, name=f"rec{it}_{j}", tag="rec", bufs=2)
            nc.vector.reciprocal_approx_fast(rec, pd[:, 512:1024])
            nc.vector.tensor_mul(ev[:, j * 512 : (j + 1) * 512], pd[:, 0:512], rec)

        def stage_O(it):
            """Wo projection (stationary-swap -> natural) + residual + stats."""
            ev, xpb_sb = evs[it], xpbs[it]
            y = work.tile([128, NTILE, D], F32, name=f"y{it}", tag=f"y{it}", bufs=1)
            mv = work.tile([128, NTILE, 2], F32, name=f"mv{it}", tag=f"mv{it}", bufs=1)
            for c in range(NTILE):
                pso = pst.tile([128, D], F32, name=f"pso{it}_{c}", tag="st")
                nc.tensor.matmul(
                    pso, ev[:, c * 128 : (c + 1) * 128], wo_sb, start=True, stop=True
                )
                nc.vector.tensor_add(y[:, c, :], pso, xpb_sb[:, c, :])
                st6 = work.tile([128, 6], F32, name=f"st6{it}_{c}", tag="st6", bufs=8)
                nc.vector.bn_stats(st6, y[:, c, :])
                nc.vector.bn_aggr(mv[:, c, :], st6)
            ys[it], mvs[it] = y, mv

        def stage_LN(it):
            """LayerNorm + store; rstd = exp(-0.5*ln(var+eps)) keeps the
            ACT table on the natural_log_exp set (no sqrt table switch)."""
            y, mv = ys[it], mvs[it]
            sd = work.tile([128, NTILE, 1], F32, name=f"sd{it}", tag="sd", bufs=2)
            nc.scalar.activation(sd, mv[:, :, 1:2], AF.Sqrt, bias=eps_sb[:, 0:1])
            rstd = work.tile([128, NTILE, 1], F32, name=f"rstd{it}", tag="rstd", bufs=2)
            nc.vector.reciprocal(rstd, sd)
            oall = work.tile([128, NTILE, D], F32, name=f"oall{it}", tag="oall", bufs=2)
            for c in range(NTILE):
                z = work.tile([128, D], F32, name=f"z{it}_{c}", tag="z", bufs=4)
                nc.vector.tensor_scalar(
                    z,
                    y[:, c, :],
                    mv[:, c, 0:1],
                    rstd[:, c, 0:1],
                    op0=mybir.AluOpType.subtract,
                    op1=mybir.AluOpType.mult,
                )
                nc.vector.tensor_mul(z, z, lng_sb)
                nc.vector.tensor_add(oall[:, c, :], z, lnb_sb)
            nc.sync.dma_start(out_d[it].rearrange("p (q d) -> p q d", q=NTILE), oall)

        # ---- half-pair software pipeline with fenced excursions ----
        stage_P(0)
        stage_A(0, 0, [lambda: stage_P(1), lambda: load_xpb(0)])
        stage_A(0, 1, [lambda: stage_B(0, 0), lambda: load_xpb(1)])
        stage_A(1, 0, [lambda: stage_B(0, 1), lambda: stage_O(0)])
        stage_A(1, 1, [lambda: stage_B(1, 0), lambda: stage_P(2, True)])
        stage_A(2, 0, [lambda: stage_B(1, 1), lambda: stage_O(1)])
        stage_A(2, 1, [lambda: stage_B(2, 0)])
        stage_B(2, 1)
        stage_O(2)
        for it in range(PER_CORE):
            stage_LN(it)

    nc.compile()
    return nc


_nc_cache = {}


def _get_nc():
    key = (POOLM, FENCE_AT)
    if key not in _nc_cache:
        _nc_cache[key] = _build_nc()
    return _nc_cache[key]


def kernel(
    node_features,
    adj_mx,
    node_type_embed,
    Wq,
    bq,
    Wk,
    bk,
    Wv,
    bv,
    edge_bias,
    Wo,
    bo,
    ln_g,
    ln_b,
):
    global LAST_RESULTS
    nf = np.asarray(node_features, np.float32)
    adj = np.asarray(adj_mx)
    nte = np.asarray(node_type_embed, np.float32)
    Wq = np.asarray(Wq, np.float32)
    Wk = np.asarray(Wk, np.float32)
    Wv = np.asarray(Wv, np.float32)
    Wo = np.asarray(Wo, np.float32)
    bq = np.asarray(bq, np.float32)
    bk = np.asarray(bk, np.float32)
    bv = np.asarray(bv, np.float32)
    bo = np.asarray(bo, np.float32)
    edge_bias = np.asarray(edge_bias, np.float32)
    ln_g = np.asarray(ln_g, np.float32)
    ln_b = np.asarray(ln_b, np.float32)

    scale = 1.0 / np.sqrt(DH)

    # shared (replicated) inputs
    types = 1 - (np.arange(N) % 2)
    ta = np.ascontiguousarray(nte[types].T)  # (D, N)
    keep = np.maximum(adj.astype(np.float32), np.eye(N, dtype=np.float32))
    mm = (np.exp(edge_bias) * keep).T.astype(BF16_NP)  # (m, nq)
    # column-doubled mask: [j0 | j0 | j1 | j1] so one [128,1024] mul covers
    # both heads of a pair (e layout is [h0 512 | h1 512] per nq half)
    maskd = np.concatenate(
        [mm[:, 0:512], mm[:, 0:512], mm[:, 512:1024], mm[:, 512:1024]], axis=1
    )
    cbf = np.concatenate(
        [
            (Wq.T * scale).astype(BF16_NP),
            Wk.T.astype(BF16_NP),
            Wv.T.astype(BF16_NP),
            Wo.T.astype(BF16_NP),
            np.ones((128, DH), BF16_NP),
        ],
        axis=1,
    )
    cf = np.concatenate(
        [
            (bq * scale).reshape(D, 1),
            bk.reshape(D, 1),
            np.broadcast_to(bv, (128, D)),
            np.broadcast_to(ln_g, (128, D)),
            np.broadcast_to(ln_b, (128, D)),
        ],
        axis=1,
    ).astype(np.float32)
    shared = {
        "ta": ta,
        "cbf": np.ascontiguousarray(cbf),
        "cf": np.ascontiguousarray(cf),
        "maskd": np.ascontiguousarray(maskd),
    }

    in_maps = []
    for c in range(NCORES):
        pairs = PAIRS[c * PER_CORE : (c + 1) * PER_CORE]
        xT = np.stack([np.ascontiguousarray(nf[b, :, t, :].T) for (b, t) in pairs])
        # [q*128+p, d] -> [p, q*d] so the device DMA is contiguous/partition
        xpb = np.stack(
            [
                (nf[b, :, t, :] + bo)
                .reshape(NTILE, 128, D)
                .transpose(1, 0, 2)
                .reshape(128, NTILE * D)
                for (b, t) in pairs
            ]
        )
        in_maps.append({**shared, "xt": xT, "xpb": np.ascontiguousarray(xpb)})

    nc = _get_nc()
    res = run_bass_kernel_spmd(
        nc,
        in_maps,
        core_ids=list(range(NCORES)),
        trace=bool(int(os.environ.get("BASSK_TRACE", "0"))),
    )
    LAST_RESULTS = res

    out = np.empty((B, N, T, D), np.float32)
    for c in range(NCORES):
        pairs = PAIRS[c * PER_CORE : (c + 1) * PER_CORE]
        for i, (b, t) in enumerate(pairs):
            o = res.results[c]["out"][i].reshape(128, NTILE, D)
            out[b, :, t, :] = o.transpose(1, 0, 2).reshape(N, D)
    return out
